# revision 1
# baseline (speedup 1.0000x reference)
"""Trainium2 Bass kernel for sparse-conv (kernel-map gather-GEMM-scatter).

Math: out[j, d] = sum over points i with out_idx[i]==j of  x[i, :] @ W[k_idx[i], :, d]

Device strategy ("dense k-slot expansion"):
  Each output voxel j owns 8 kernel-offset slots (k = 0..7); each active input
  point occupies exactly slot (j=out_idx[i], k=k_idx[i]) (unique by construction
  for stride-2/kernel-2 sparse conv). Host scatters x rows into a dense
  xgT[k*32+c, j] tensor; then  out.T = Wflat.T @ xgT  is one plain GEMM with
  K=256 contraction — the device does zero gather/scatter, just DMA + TensorE.
  Output voxels are sharded contiguously across the 8 cores (device-local
  output partitions => no collective needed).

Layout per core (S segs, S multiple of 8192):
  in : xgT  [256, S] (two K-halves of 128 partitions), wflat [256, 32]
  out: out_st [S/8192, 128, 2048]; element [b, 32a+d, 512g+t] holds
       out.T[d, seg] with seg = b*8192 + g*2048 + a*512 + t
"""
import sys

if "/opt/trn_rl_repo" not in sys.path:
    sys.path.insert(0, "/opt/trn_rl_repo")

import numpy as np

N_CORES = 8
BLK = 8192          # segs per staging block ( [128, 2048] staging tile )
DT_IN = "float32"   # dtype of xgT/wflat on device ("float32" or "bfloat16")

_prog_cache = {}


def _build_program(S, dt_name):
    import concourse.tile as tile
    from concourse import bacc, mybir

    dt = getattr(mybir.dt, dt_name)
    f32 = mybir.dt.float32
    nc = bacc.Bacc("TRN2", target_bir_lowering=False, debug=False)
    xgT_d = nc.dram_tensor("xgT", [256, S], dt, kind="ExternalInput")
    w_d = nc.dram_tensor("wflat", [256, 32], dt, kind="ExternalInput")
    nb = S // BLK
    out_d = nc.dram_tensor("out_st", [nb, 128, 2048], f32, kind="ExternalOutput")

    with tile.TileContext(nc) as tc:
        with (
            tc.tile_pool(name="w", bufs=1) as wpool,
            tc.tile_pool(name="xin", bufs=5) as xpool,
            tc.tile_pool(name="st", bufs=2) as stpool,
            tc.tile_pool(name="ps", bufs=8, space="PSUM") as pspool,
        ):
            w0 = wpool.tile([128, 32], dt, tag="w0")
            w1 = wpool.tile([128, 32], dt, tag="w1")
            nc.sync.dma_start(w0[:], w_d.ap()[0:128, :])
            nc.scalar.dma_start(w1[:], w_d.ap()[128:256, :])

            for b in range(nb):
                staging = stpool.tile([128, 2048], f32)
                for g in range(4):
                    seg0 = b * BLK + g * 2048
                    x0 = xpool.tile([128, 2048], dt, tag="x0")
                    x1 = xpool.tile([128, 2048], dt, tag="x1")
                    nc.sync.dma_start(x0[:], xgT_d.ap()[0:128, seg0:seg0 + 2048])
                    nc.scalar.dma_start(x1[:], xgT_d.ap()[128:256, seg0:seg0 + 2048])
                    for a in range(4):
                        ps = pspool.tile([32, 512], f32)
                        nc.tensor.matmul(ps[:], w0[:], x0[:, 512 * a:512 * (a + 1)],
                                         start=True, stop=False)
                        nc.tensor.matmul(ps[:], w1[:], x1[:, 512 * a:512 * (a + 1)],
                                         start=False, stop=True)
                        eng = nc.vector if (a % 2 == 0) else nc.scalar
                        dst = staging[32 * a:32 * (a + 1), 512 * g:512 * (g + 1)]
                        if eng is nc.vector:
                            eng.tensor_copy(dst, ps[:])
                        else:
                            eng.copy(dst, ps[:])
                nc.gpsimd.dma_start(out_d.ap()[b], staging[:])

    nc.compile()
    return nc


def _get_program(S, dt_name):
    key = (S, dt_name)
    if key not in _prog_cache:
        _prog_cache[key] = _build_program(S, dt_name)
    return _prog_cache[key]


def _pack(x, W, k_idx, out_idx, num_out, dt_np):
    """Host-side: scatter x into dense k-slot layout, per-core [256, S] slabs."""
    n = x.shape[0]
    S = -(-num_out // (N_CORES * BLK)) * BLK  # per-core segs, padded
    Stot = N_CORES * S

    xg4 = np.zeros((Stot, 8, 32), dtype=np.float32)
    pairs = out_idx.astype(np.int64) * 8 + k_idx
    if np.unique(pairs).size == n:
        xg4[out_idx, k_idx] = x
    else:  # duplicate (voxel, offset) pairs: accumulate
        np.add.at(xg4, (out_idx, k_idx), x)

    wflat = W.reshape(256, 32).astype(dt_np)
    in_maps = []
    for c in range(N_CORES):
        slab = xg4[c * S:(c + 1) * S].reshape(S, 256).T  # [256, S]
        in_maps.append({
            "xgT": np.ascontiguousarray(slab).astype(dt_np, copy=False),
            "wflat": wflat,
        })
    return in_maps, S


def _decode(results, S, num_out):
    """Per-core out_st [nb,128,2048] -> out [num_out, 32]."""
    outs = []
    for r in results:
        st = r["out_st"]  # [nb, 128, 2048]
        nb = st.shape[0]
        arr = st.reshape(nb, 4, 32, 4, 512)          # [b, a, d, g, t]
        outT = arr.transpose(2, 0, 3, 1, 4).reshape(32, S)  # [d, seg]
        outs.append(outT.T)                           # [S, 32]
    full = np.concatenate(outs, axis=0)
    return np.ascontiguousarray(full[:num_out])


def run(x, W, k_idx, out_idx, num_out, trace=False, dt_name=DT_IN):
    from concourse.bass_utils import run_bass_kernel_spmd

    x = np.asarray(x, dtype=np.float32)
    W = np.asarray(W, dtype=np.float32)
    k_idx = np.asarray(k_idx, dtype=np.int32)
    out_idx = np.asarray(out_idx, dtype=np.int32)
    num_out = int(num_out)

    if dt_name == "bfloat16":
        import ml_dtypes
        dt_np = ml_dtypes.bfloat16
    else:
        dt_np = {"float32": np.float32, "float16": np.float16}[dt_name]

    in_maps, S = _pack(x, W, k_idx, out_idx, num_out, dt_np)
    nc = _get_program(S, dt_name)
    res = run_bass_kernel_spmd(nc, in_maps, list(range(N_CORES)), trace=trace)
    out = _decode(res.results, S, num_out)
    return out, res


def kernel(x, W, k_idx, out_idx, num_out):
    out, _ = run(x, W, k_idx, out_idx, num_out, trace=False)
    return out



# revision 12
# speedup vs baseline: 1.7552x; 1.7552x over previous
"""Trainium2 Bass kernel for sparse-conv (kernel-map gather-GEMM-scatter).

Math: out[j, d] = sum over points i with out_idx[i]==j of  x[i, :] @ W[k_idx[i], :, d]

Strategy ("exact k-set class GEMMs", zero-free packing):
  Each output voxel j owns the set S_j of kernel offsets its points occupy.
  Voxels are grouped into classes by that exact set (voxels with |S|>4 are
  split into two stream-voxels of <=4 offsets each; the host adds the two
  halves back together).  For a class with set S (|S|=c), the per-voxel
  contribution is a single K=32c GEMM against the stacked weight
  [W[S_0]; ...; W[S_{c-1}]]  -- so the moving operand contains ONLY real
  point data (no dense 8-slot zero fill; ~2.6x less DMA).

  Device layout: class rectangles (height 32c, width = #voxels of that class
  on this core) are shelf-packed into a single [128, F] bf16 slab; a class at
  row offset 32r runs as matmul tile_position=(32r, 32g) writing PSUM
  partitions [32g, 32g+32) -- 4 column-groups pack 4x512 voxel outputs into
  one [128, 512] PSUM bank.  PSUM -> SBUF (bf16 cast) on DVE/ACT, SWDGE DMA
  out on GpSimd.

  Stream-voxels are dealt round-robin across the 8 cores so every core has an
  identical class-count vector => one SPMD program for all cores.  The
  program structure depends only on the per-class counts (cached by key).
"""
import sys

if "/opt/trn_rl_repo" not in sys.path:
    sys.path.insert(0, "/opt/trn_rl_repo")

import numpy as np

N_CORES = 8
PSUM_N = 512          # psum bank columns (f32)
TILE_SLOTS = 2048     # 4 col-groups x 512 per psum tile
STAGE_SLOTS = 8192    # 4 psum tiles per [128, 2048] staging tile
CHUNK_COLS = 3072     # xslab DMA chunk granularity (columns)
DT_IN = "bfloat16"    # kept for test.py compat

# ---------------------------------------------------------------- mask tables
_PC = np.array([bin(m).count("1") for m in range(256)], dtype=np.int64)


def _split_mask(m):
    if _PC[m] <= 4:
        return m, 0
    bits = [b for b in range(8) if (m >> b) & 1]
    a = sum(1 << b for b in bits[:4])
    return a, m - a


_AMASK = np.array([_split_mask(m)[0] for m in range(256)], dtype=np.int64)
_BMASK = np.array([_split_mask(m)[1] for m in range(256)], dtype=np.int64)
_RANK = np.zeros((256, 8), dtype=np.int64)
for _m in range(256):
    _r = 0
    for _k in range(8):
        if (_m >> _k) & 1:
            _RANK[_m, _k] = _r
            _r += 1

_prog_cache = {}


# ------------------------------------------------------------------- planning
def _plan(x, W, k_idx, out_idx, num_out):
    """Returns (layout, meta, xslabs, wslab). layout drives program build."""
    n = x.shape[0]
    vox = out_idx.astype(np.int64)
    kk = k_idx.astype(np.int64)
    xv = x

    # guard out-of-range (reference segment_sum drops them)
    if vox.max(initial=0) >= num_out or vox.min(initial=0) < 0:
        keep = (vox >= 0) & (vox < num_out)
        vox, kk, xv = vox[keep], kk[keep], xv[keep]

    # pre-accumulate duplicate (voxel, offset) pairs
    pair = vox * 8 + kk
    order0 = np.argsort(pair, kind="stable")
    ps_sorted = pair[order0]
    uniq = np.ones(len(ps_sorted), bool)
    if len(ps_sorted) > 1:
        uniq[1:] = ps_sorted[1:] != ps_sorted[:-1]
    if not uniq.all():
        grp = np.cumsum(uniq) - 1
        m = int(grp[-1]) + 1
        xa = np.zeros((m, xv.shape[1]), np.float32)
        np.add.at(xa, grp, xv[order0])
        pu = ps_sorted[uniq]
        vox, kk, xv = pu // 8, pu % 8, xa

    # per-voxel k-set bitmask (sums of distinct powers of two == OR)
    mask = np.bincount(vox, weights=2.0 ** kk,
                       minlength=num_out).astype(np.int64)
    amask = _AMASK[mask]
    bmask = _BMASK[mask]

    avox = np.nonzero(mask)[0]
    bvox = np.nonzero(bmask)[0]
    sv_vox = np.concatenate([avox, bvox])
    sv_mask = np.concatenate([amask[avox], bmask[bvox]])
    sv_isb = np.concatenate([np.zeros(len(avox), np.int64),
                             np.ones(len(bvox), np.int64)])
    nsv = len(sv_vox)

    svo = np.argsort(sv_mask, kind="stable")
    sv_vox_s = sv_vox[svo]
    sv_mask_s = sv_mask[svo]
    sv_isb_s = sv_isb[svo]

    cls_masks, cls_starts, cls_counts = np.unique(
        sv_mask_s, return_index=True, return_counts=True)
    j_in_cls = np.arange(nsv) - np.repeat(cls_starts, cls_counts)
    sv_core = j_in_cls % N_CORES
    sv_col = j_in_cls // N_CORES

    # ---- psum layout + per-segment packing (identical on all cores) -----
    # HW quirk: two matmuls with the same rounded tile K but different row
    # offsets hitting the same PSUM col-group crash the NEFF.  Workaround:
    # the row offset is a fixed function of (col-group, K):
    #   h=1 (K=32):  r = g;   h=2 (K=64): r = 2*(g%2);   h>=3: r = 0
    C = -(-cls_counts // N_CORES)          # cols per class per core
    H = _PC[cls_masks]
    order_c = np.argsort(-H, kind="stable")

    def _row_of(h, g):
        if h == 1:
            return g
        if h == 2:
            return 2 * (g % 2)
        return 0

    lane = [0, 0, 0, 0]    # xslab lane cursors
    wlane = [0, 0, 0, 0]   # weight slab lane cursors (32-col block units)
    wblocks = {}           # (mask, r) -> wcol
    cls_q0 = {}            # mask -> psum start
    cls_segs = {}          # mask -> list of (seg_start_in_cls, nw, g, r, col0)
    q = 0

    def _walloc(mask, h, r):
        kb = (mask, r)
        if kb not in wblocks:
            wc = max(wlane[r:r + h])
            for rr in range(r, r + h):
                wlane[rr] = wc + 1
            wblocks[kb] = wc * 32
        return wblocks[kb]

    for ci in order_c:
        mk = int(cls_masks[ci])
        h = int(H[ci])
        cc = int(C[ci])
        cls_q0[mk] = q
        segs = []
        s = q
        while s < q + cc:
            s1 = min(q + cc, (s // PSUM_N + 1) * PSUM_N)
            g = (s % TILE_SLOTS) // PSUM_N
            r = _row_of(h, g)
            nw = s1 - s
            col0 = max(lane[r:r + h])
            if col0 // CHUNK_COLS != (col0 + nw - 1) // CHUNK_COLS:
                col0 = ((col0 // CHUNK_COLS) + 1) * CHUNK_COLS
            for rr in range(r, r + h):
                lane[rr] = col0 + nw
            segs.append((s - q, nw, g, r, col0))
            s = s1
        cls_segs[mk] = segs
        q += cc

    # dedicated pad weight blocks (zero weights) per row, K=32
    pad_w = [_walloc((-1, g), 1, g) for g in range(4)]

    # zero pad block so every slot of every used psum tile gets a matmul
    F0 = -(-max(lane) // 64) * 64
    if F0 % CHUNK_COLS > CHUNK_COLS - PSUM_N:
        F0 = (F0 // CHUNK_COLS + 1) * CHUNK_COLS
    pad_base = F0
    F = F0 + PSUM_N
    Q = q
    n_ptiles = -(-Q // TILE_SLOTS)
    nstage = -(-n_ptiles // 4)
    nchunk = -(-F // CHUNK_COLS)
    chunks = [(i * CHUNK_COLS, min((i + 1) * CHUNK_COLS, F))
              for i in range(nchunk)]

    # ---- matmul segment list per psum tile ------------------------------
    # segment: (group, c0, c1, xr, h, wr, wcol, chunk_i, xlo)
    tile_mms = [[[] for _ in range(4)] for _ in range(n_ptiles)]
    for ci in order_c:
        mk = int(cls_masks[ci])
        h = int(H[ci])
        q0 = cls_q0[mk]
        for (off, nw, g, r, col0) in cls_segs[mk]:
            s = q0 + off
            pt = s // TILE_SLOTS
            c0 = s % PSUM_N
            wcol = _walloc(mk, h, r)
            chunk_i = col0 // CHUNK_COLS
            xlo = col0 - chunk_i * CHUNK_COLS
            tile_mms[pt][g].append(
                (g, c0, c0 + nw, r, h, r, wcol, chunk_i, xlo))
    # pad matmuls over the zero block (zero weights x zero data)
    s = Q
    pad_ci = pad_base // CHUNK_COLS
    pad_xlo = pad_base - pad_ci * CHUNK_COLS
    while s < n_ptiles * TILE_SLOTS:
        s1 = min(n_ptiles * TILE_SLOTS, (s // PSUM_N + 1) * PSUM_N)
        pt = s // TILE_SLOTS
        g = (s % TILE_SLOTS) // PSUM_N
        c0 = s % PSUM_N
        tile_mms[pt][g].append(
            (g, c0, c0 + (s1 - s), g, 1, g, pad_w[g], pad_ci, pad_xlo))
        s = s1

    WF = max(max(wlane) * 32, 32)

    layout = {
        "F": F, "WF": WF, "Q": Q, "n_ptiles": n_ptiles, "nstage": nstage,
        "chunks": tuple(chunks),
        "tile_mms": tuple(tuple(tuple(g) for g in t) for t in tile_mms),
    }
    key = (F, WF, n_ptiles, nstage, layout["chunks"], layout["tile_mms"])

    # ---- per-sv placement (segment-aware) -------------------------------
    import ml_dtypes
    bf16 = ml_dtypes.bfloat16

    info_q0 = np.array([cls_q0[int(m)] for m in cls_masks])
    cls_of_mask = np.zeros(256, np.int64)
    cls_of_mask[cls_masks] = np.arange(len(cls_masks))

    sv_r = np.zeros(nsv, np.int64)       # lane row of the sv's segment
    sv_xcol = np.zeros(nsv, np.int64)    # xslab column of the sv
    for ci in range(len(cls_masks)):
        mk = int(cls_masks[ci])
        sl = slice(cls_starts[ci], cls_starts[ci] + cls_counts[ci])
        colc_l = sv_col[sl]
        segs = cls_segs[mk]
        offs = np.array([o for (o, nw, g, r, c0) in segs])
        nws = np.array([nw for (o, nw, g, r, c0) in segs])
        rs = np.array([r for (o, nw, g, r, c0) in segs])
        c0s = np.array([c0 for (o, nw, g, r, c0) in segs])
        si = np.searchsorted(offs, colc_l, side="right") - 1
        sv_r[sl] = rs[si]
        sv_xcol[sl] = c0s[si] + (colc_l - offs[si])

    # point -> stream-voxel (sorted index)
    inA = ((amask[vox] >> kk) & 1).astype(np.int64)
    pkey = vox * 2 + (1 - inA)
    sv_key_s = sv_vox_s * 2 + sv_isb_s
    ks = np.argsort(sv_key_s, kind="stable")
    pos = ks[np.searchsorted(sv_key_s[ks], pkey)]
    pm = sv_mask_s[pos]
    slot = _RANK[pm, kk]
    p_core = sv_core[pos]
    p_col = sv_xcol[pos]
    p_rb = sv_r[pos] + slot

    xslab = np.zeros((N_CORES, 4, 32, F), np.float32)
    xslab[p_core, p_rb, :, p_col] = xv
    xslab = xslab.reshape(N_CORES, 128, F).astype(bf16)

    # ---- weight slab (same for all cores) -------------------------------
    wslab = np.zeros((4, 32, WF), np.float32)
    for (mk, r), wcol in wblocks.items():
        if not isinstance(mk, (int, np.integer)):
            continue   # pad block stays zero
        h = int(_PC[mk])
        bits = [b for b in range(8) if (mk >> b) & 1]
        st = np.stack([W[b] for b in bits])          # [h, 32, 32]
        for s in range(h):
            wslab[r + s, :, wcol:wcol + 32] = st[s]
    wslab = wslab.reshape(128, WF).astype(bf16)

    # ---- decode metadata -------------------------------------------------
    sv_slot = info_q0[cls_of_mask[sv_mask_s]] + sv_col
    meta = {
        "nstage": nstage, "num_out": num_out,
        "sv_core": sv_core, "sv_slot": sv_slot,
        "sv_vox": sv_vox_s, "sv_isb": sv_isb_s,
    }
    return key, layout, meta, xslab, wslab


# ------------------------------------------------------------ device program
def _build_program(key, layout):
    import concourse.tile as tile
    from concourse import bacc, mybir

    bf16 = mybir.dt.bfloat16
    f32 = mybir.dt.float32
    F, WF = layout["F"], layout["WF"]
    n_ptiles, nstage = layout["n_ptiles"], layout["nstage"]
    chunks = layout["chunks"]
    tile_mms = layout["tile_mms"]

    nc = bacc.Bacc("TRN2", target_bir_lowering=False, debug=False)
    x_d = nc.dram_tensor("xslab", [128, F], bf16, kind="ExternalInput")
    w_d = nc.dram_tensor("wslab", [128, WF], bf16, kind="ExternalInput")
    out_d = nc.dram_tensor("out_st", [nstage, 128, 2048], bf16,
                           kind="ExternalOutput")

    with tile.TileContext(nc) as tc:
        with (
            tc.tile_pool(name="w", bufs=1) as wpool,
            tc.tile_pool(name="xin", bufs=1) as xpool,
            tc.tile_pool(name="st", bufs=3) as stpool,
            tc.tile_pool(name="ps", bufs=8, space="PSUM") as pspool,
        ):
            w = wpool.tile([128, WF], bf16, tag="w")
            nc.sync.dma_start(w[:], w_d.ap())

            xt = []
            for i, (lo, hi) in enumerate(chunks):
                t = xpool.tile([128, hi - lo], bf16, tag=f"x{i}")
                eng = nc.sync if i % 2 == 0 else nc.scalar
                eng.dma_start(t[:], x_d.ap()[:, lo:hi])
                xt.append(t)

            for stage in range(nstage):
                stg = stpool.tile([128, 2048], bf16, tag="stg")
                n_valid = min(4, n_ptiles - stage * 4)
                for p in range(n_valid):
                    pt = stage * 4 + p
                    ps = pspool.tile([128, PSUM_N], f32, tag="ps")
                    # interleave issue across the 4 col-groups
                    groups = [list(g) for g in tile_mms[pt]]
                    while any(groups):
                        for g in range(4):
                            if groups[g]:
                                (gg, c0, c1, r, h, wr, wcol, ci, xlo) = \
                                    groups[g].pop(0)
                                nw = c1 - c0
                                nc.tensor.matmul(
                                    ps[32 * gg:32 * gg + 32, c0:c1],
                                    w[32 * wr:32 * (wr + h), wcol:wcol + 32],
                                    xt[ci][32 * r:32 * (r + h), xlo:xlo + nw],
                                    start=True, stop=True,
                                    tile_position=(32 * r, 32 * gg))
                    eng = nc.vector if p % 2 == 0 else nc.scalar
                    if eng is nc.vector:
                        eng.tensor_copy(stg[:, 512 * p:512 * (p + 1)], ps[:])
                    else:
                        eng.copy(stg[:, 512 * p:512 * (p + 1)], ps[:])
                nc.gpsimd.dma_start(out_d.ap()[stage][:, :512 * n_valid],
                                    stg[:, :512 * n_valid])

    nc.compile()
    return nc


def _get_program(key, layout):
    if key not in _prog_cache:
        _prog_cache[key] = _build_program(key, layout)
    return _prog_cache[key]


# -------------------------------------------------------------------- decode
def _decode(results, meta):
    nstage = meta["nstage"]
    num_out = meta["num_out"]
    per_core = []
    for r in results:
        st = np.asarray(r["out_st"]).astype(np.float32)   # [nstage,128,2048]
        arr = st.reshape(nstage, 4, 32, 4, 512)           # [s, g, ch, p, col]
        arr = arr.transpose(0, 3, 1, 4, 2).reshape(-1, 32)  # (s,p,g,col),ch
        per_core.append(arr)
    stacked = np.stack(per_core)                          # [8, slots, 32]
    vals = stacked[meta["sv_core"], meta["sv_slot"]]      # [nsv, 32]
    out = np.zeros((num_out, 32), np.float32)
    isb = meta["sv_isb"].astype(bool)
    out[meta["sv_vox"][~isb]] = vals[~isb]
    np.add.at(out, meta["sv_vox"][isb], vals[isb])
    return out


# ---------------------------------------------------------------------- main
def run(x, W, k_idx, out_idx, num_out, trace=False, dt_name=DT_IN):
    from concourse.bass_utils import run_bass_kernel_spmd

    x = np.asarray(x, dtype=np.float32)
    W = np.asarray(W, dtype=np.float32)
    k_idx = np.asarray(k_idx, dtype=np.int32)
    out_idx = np.asarray(out_idx, dtype=np.int32)
    num_out = int(num_out)

    key, layout, meta, xslab, wslab = _plan(x, W, k_idx, out_idx, num_out)
    nc = _get_program(key, layout)
    in_maps = [{"xslab": np.ascontiguousarray(xslab[c]), "wslab": wslab}
               for c in range(N_CORES)]
    res = run_bass_kernel_spmd(nc, in_maps, list(range(N_CORES)), trace=trace)
    out = _decode(res.results, meta)
    return out, res


def kernel(x, W, k_idx, out_idx, num_out):
    out, _ = run(x, W, k_idx, out_idx, num_out, trace=False)
    return out
